# revision 4
# baseline (speedup 1.0000x reference)
"""Trainium2 Bass kernel for the per-element MLP problem.

Computes out = x + MLP(x) where MLP is a tiny per-element 1->8->8->1
network with ReLUs:
    h0 = relu(x * W0 + b0)      # (N, 8)
    h1 = relu(h0 @ W1 + b1)     # (N, 8)
    y  = h1 @ W2 + b2           # (N, 1)
    out = y + x

Strategy (per core, pure data parallel over 8 cores):
  - Core's 4Mi-element slice viewed as [128, 32768] fp32 in SBUF tiles
    of [128, 2048], processed in 512-column blocks.
  - Features are evaluated on the PE array with block-structured
    stationary matrices, 16 elements per PE column:
      mm1(i): lhsT W0map_i maps x-streams 16i..16i+15 (of the full
              [128, 512] x block) onto 128 layer-0 pre-activations
              (16 groups x 8 features).
      act1:   relu(psum + b0vec) -> h0   (ScalarE/VectorE alternating)
      mm2:    block-diag(W1) -> layer-1 pre-activations in PSUM
      act2:   relu(psum + b1vec) -> h1
      mm3(i): W2map_i contracts each group's 8 features back to its
              element's scalar y, accumulated at PSUM partition 16i+g
              so after 8 i-blocks the PSUM bank holds y in the SAME
              [128, 512] layout as the x block.
  - Final: out = (y_psum + b2) + x via one VectorE scalar_tensor_tensor
    (exact fp32 residual add), then DMA out.
  - All matmuls use float32r (full-rate on the PE) with K=128, N=512,
    base partition 0.
"""

import os
from contextlib import ExitStack

import numpy as np


import concourse.bacc as bacc
import concourse.tile as tile
from concourse import mybir
from concourse.bass_utils import run_bass_kernel_spmd

N_CORES = 8
FULL_SHAPE = (8, 1024, 4096)
TOTAL = FULL_SHAPE[0] * FULL_SHAPE[1] * FULL_SHAPE[2]
PER_CORE = TOTAL // N_CORES  # 4194304
P = 128
FREE = PER_CORE // P  # 32768
CTILE = 2048  # columns per supertile (one DMA)
NSUPER = FREE // CTILE  # 16
NBLK = CTILE // 512  # 512-column blocks per supertile

F32 = mybir.dt.float32
F32R = mybir.dt.float32r
RELU = mybir.ActivationFunctionType.Relu
ADD = mybir.AluOpType.add
MAX = mybir.AluOpType.max

_BUILD_CACHE = {}


def _build_nc(nsuper: int):
    """Build the per-core Bass program. Weight values are runtime inputs,
    so the compiled program is reusable for any weights."""
    free = nsuper * CTILE
    nc = bacc.Bacc("TRN2", target_bir_lowering=False, debug=False)

    x_d = nc.dram_tensor("x", [P, free], F32R, kind="ExternalInput").ap()
    w0m_d = nc.dram_tensor("w0maps", [P, 8 * 128], F32R, kind="ExternalInput").ap()
    w1b_d = nc.dram_tensor("w1blk", [P, 128], F32R, kind="ExternalInput").ap()
    w2m_d = nc.dram_tensor("w2maps", [P, 8 * 128], F32R, kind="ExternalInput").ap()
    bv_d = nc.dram_tensor("biasvecs", [P, 3], F32, kind="ExternalInput").ap()
    out_d = nc.dram_tensor("out", [P, free], F32, kind="ExternalOutput").ap()

    with tile.TileContext(nc) as tc, ExitStack() as ctx:
        consts = ctx.enter_context(tc.tile_pool(name="consts", bufs=1))
        xpool = ctx.enter_context(tc.tile_pool(name="xin", bufs=3))
        opool = ctx.enter_context(tc.tile_pool(name="oout", bufs=3))
        h0pool = ctx.enter_context(tc.tile_pool(name="h0", bufs=3))
        h1pool = ctx.enter_context(tc.tile_pool(name="h1", bufs=3))
        apool = ctx.enter_context(tc.tile_pool(name="psA", bufs=2, space="PSUM"))
        bpool = ctx.enter_context(tc.tile_pool(name="psB", bufs=2, space="PSUM"))
        ypool = ctx.enter_context(tc.tile_pool(name="psY", bufs=2, space="PSUM"))

        w0m = consts.tile([P, 8 * 128], F32R)
        w1b = consts.tile([P, 128], F32R)
        w2m = consts.tile([P, 8 * 128], F32R)
        bv = consts.tile([P, 3], F32)
        nc.sync.dma_start(w0m[:], w0m_d[:])
        nc.sync.dma_start(w1b[:], w1b_d[:])
        nc.sync.dma_start(w2m[:], w2m_d[:])
        nc.sync.dma_start(bv[:], bv_d[:])
        b0vec = bv[:, 0:1]
        b1vec = bv[:, 1:2]
        b2vec = bv[:, 2:3]

        actsel = 0
        for s in range(nsuper):
            x_sb = xpool.tile([P, CTILE], F32R)
            nc.sync.dma_start(x_sb[:], x_d[:, s * CTILE : (s + 1) * CTILE])
            out_sb = opool.tile([P, CTILE], F32)
            for b in range(NBLK):
                c0 = b * 512
                x_blk_r = x_sb[:, c0 : c0 + 512]
                x_blk_f32 = x_blk_r.bitcast(F32)
                y_ps = ypool.tile([P, 512], F32)
                for i in range(8):
                    a_ps = apool.tile([P, 512], F32)
                    nc.tensor.matmul(
                        a_ps[:],
                        w0m[:, 128 * i : 128 * (i + 1)],
                        x_blk_r,
                        start=True,
                        stop=True,
                    )
                    h0 = h0pool.tile([P, 512], F32R)
                    if actsel % 2 == 0:
                        nc.scalar.activation(h0[:], a_ps[:], RELU, bias=b0vec)
                    else:
                        nc.vector.tensor_scalar(
                            h0[:], a_ps[:], b0vec, 0.0, op0=ADD, op1=MAX
                        )
                    z_ps = bpool.tile([P, 512], F32)
                    nc.tensor.matmul(
                        z_ps[:],
                        w1b[:],
                        h0[:],
                        start=True,
                        stop=True,
                    )
                    h1 = h1pool.tile([P, 512], F32R)
                    if actsel % 2 == 0:
                        nc.vector.tensor_scalar(
                            h1[:], z_ps[:], b1vec, 0.0, op0=ADD, op1=MAX
                        )
                    else:
                        nc.scalar.activation(h1[:], z_ps[:], RELU, bias=b1vec)
                    actsel += 1
                    nc.tensor.matmul(
                        y_ps[:],
                        w2m[:, 128 * i : 128 * (i + 1)],
                        h1[:],
                        start=(i == 0),
                        stop=(i == 7),
                    )
                # out = (y + b2) + x  -- exact fp32 residual add on VectorE
                nc.vector.scalar_tensor_tensor(
                    out_sb[:, c0 : c0 + 512],
                    y_ps[:],
                    b2vec,
                    x_blk_f32,
                    op0=ADD,
                    op1=ADD,
                )
            nc.sync.dma_start(out_d[:, s * CTILE : (s + 1) * CTILE], out_sb[:])

    nc.compile()
    return nc


def _get_nc(nsuper: int):
    if nsuper not in _BUILD_CACHE:
        _BUILD_CACHE[nsuper] = _build_nc(nsuper)
    return _BUILD_CACHE[nsuper]


def _make_weight_maps(W0, b0, W1, b1, W2, b2):
    """Host-side packing of the block stationary matrices."""
    W0v = np.asarray(W0, np.float32).reshape(8)
    b0v = np.asarray(b0, np.float32).reshape(8)
    W1m = np.asarray(W1, np.float32).reshape(8, 8)
    b1v = np.asarray(b1, np.float32).reshape(8)
    W2v = np.asarray(W2, np.float32).reshape(8)
    b2val = float(np.asarray(b2, np.float32).reshape(()))

    w0maps = np.zeros((P, 8 * 128), np.float32)
    w2maps = np.zeros((P, 8 * 128), np.float32)
    for i in range(8):
        for g in range(16):
            for j in range(8):
                # mm1: a[8g+j] += W0[j] * x[16i+g]
                w0maps[16 * i + g, 128 * i + 8 * g + j] = W0v[j]
                # mm3: y[16i+g] += W2[j] * h1[8g+j]
                w2maps[8 * g + j, 128 * i + 16 * i + g] = W2v[j]

    w1blk = np.zeros((P, 128), np.float32)
    for g in range(16):
        # mm2: z[8g+k] += W1[j,k] * h0[8g+j]  (lhsT[row=j_in, col=k_out])
        w1blk[8 * g : 8 * g + 8, 8 * g : 8 * g + 8] = W1m

    biasvecs = np.zeros((P, 3), np.float32)
    biasvecs[:, 0] = np.tile(b0v, 16)
    biasvecs[:, 1] = np.tile(b1v, 16)
    biasvecs[:, 2] = b2val
    return w0maps, w1blk, w2maps, biasvecs


def _run(x, W0, b0, W1, b1, W2, b2, nsuper=NSUPER, trace=False):
    x = np.ascontiguousarray(np.asarray(x, np.float32))
    w0maps, w1blk, w2maps, biasvecs = _make_weight_maps(W0, b0, W1, b1, W2, b2)

    free = nsuper * CTILE
    n_el = N_CORES * P * free
    xs = x.reshape(-1)[:n_el].reshape(N_CORES, P, free)

    nc = _get_nc(nsuper)
    in_maps = [
        {
            "x": np.ascontiguousarray(xs[c]),
            "w0maps": w0maps,
            "w1blk": w1blk,
            "w2maps": w2maps,
            "biasvecs": biasvecs,
        }
        for c in range(N_CORES)
    ]
    res = run_bass_kernel_spmd(nc, in_maps, list(range(N_CORES)), trace=trace)
    out = np.stack([res.results[c]["out"] for c in range(N_CORES)])
    return out, res


def kernel(x, W0, b0, W1, b1, W2, b2):
    out, _ = _run(x, W0, b0, W1, b1, W2, b2)
    return out.reshape(FULL_SHAPE).astype(np.float32)


# revision 5
# speedup vs baseline: 7.2569x; 7.2569x over previous
"""Trainium2 Bass kernel for the per-element MLP problem.

Computes out = x + MLP(x) where MLP is a tiny per-element 1->8->8->1
network with ReLUs:
    h0 = relu(x * W0 + b0)      # (N, 8)
    h1 = relu(h0 @ W1 + b1)     # (N, 8)
    y  = h1 @ W2 + b2           # (N, 1)
    out = y + x

Strategy (per core, pure data parallel over 8 cores):
  - Core's 4Mi-element slice viewed as [128, 32768] fp32, DMA'd in
    [128, 2048] supertiles, processed as 1024-column pairs of
    512-column blocks.
  - Feature mixing runs on the PE array with block-structured
    stationary matrices, 16 elements per PE column:
      mm1(i): W0map_i (fp32r) maps x-streams 16i..16i+15 of the full
              [128, 512] x block onto 128 layer-0 pre-activations
              (16 groups x 8 features).
      act1:   relu(psum + b0vec) -> h0 (bf16), FD=1024 spanning both
              blocks of the pair (ScalarE/VectorE alternating)
      mm2:    block-diag(W1) (bf16) -> layer-1 pre-acts in PSUM
      act2:   relu(psum + b1vec) -> h1 (bf16)
      mm3(i): W2map_i (bf16) contracts each group's 8 features back to
              its element's scalar y, accumulated at PSUM partition
              16i+g: after 8 i-blocks the PSUM pair holds y in the
              SAME [128, 1024] layout as the x pair.
  - Final: out = (y_psum + b2) + x via one VectorE
    scalar_tensor_tensor at FD=1024 (exact fp32 residual add).
  - PSUM budget: A-pair x2 (4 banks) + B-pair x1 (2) + Y-pair x1 (2).
"""

from contextlib import ExitStack

import numpy as np

import concourse.bacc as bacc
import concourse.tile as tile
from concourse import mybir
from concourse.bass_utils import run_bass_kernel_spmd

N_CORES = 8
FULL_SHAPE = (8, 1024, 4096)
TOTAL = FULL_SHAPE[0] * FULL_SHAPE[1] * FULL_SHAPE[2]
PER_CORE = TOTAL // N_CORES  # 4194304
P = 128
FREE = PER_CORE // P  # 32768
CTILE = 2048  # columns per supertile (one DMA)
NSUPER = FREE // CTILE  # 16
NPAIR = CTILE // 1024  # 1024-column pairs per supertile

F32 = mybir.dt.float32
F32R = mybir.dt.float32r
BF16 = mybir.dt.bfloat16
RELU = mybir.ActivationFunctionType.Relu
ADD = mybir.AluOpType.add
MAX = mybir.AluOpType.max

_BUILD_CACHE = {}


def _build_nc(nsuper: int):
    """Build the per-core Bass program. Weight values are runtime inputs,
    so the compiled program is reusable for any weights."""
    free = nsuper * CTILE
    nc = bacc.Bacc("TRN2", target_bir_lowering=False, debug=False)

    x_d = nc.dram_tensor("x", [P, free], F32R, kind="ExternalInput").ap()
    w0m_d = nc.dram_tensor("w0maps", [P, 8 * 128], F32R, kind="ExternalInput").ap()
    w1b_d = nc.dram_tensor("w1blk", [P, 128], BF16, kind="ExternalInput").ap()
    w2m_d = nc.dram_tensor("w2maps", [P, 8 * 128], BF16, kind="ExternalInput").ap()
    bv_d = nc.dram_tensor("biasvecs", [P, 3], F32, kind="ExternalInput").ap()
    out_d = nc.dram_tensor("out", [P, free], F32, kind="ExternalOutput").ap()

    with tile.TileContext(nc) as tc, ExitStack() as ctx:
        consts = ctx.enter_context(tc.tile_pool(name="consts", bufs=1))
        xpool = ctx.enter_context(tc.tile_pool(name="xin", bufs=3))
        opool = ctx.enter_context(tc.tile_pool(name="oout", bufs=3))
        h0pool = ctx.enter_context(tc.tile_pool(name="h0", bufs=3))
        h1pool = ctx.enter_context(tc.tile_pool(name="h1", bufs=3))
        apool = ctx.enter_context(tc.tile_pool(name="psA", bufs=2, space="PSUM"))
        bpool = ctx.enter_context(tc.tile_pool(name="psB", bufs=1, space="PSUM"))
        ypool = ctx.enter_context(tc.tile_pool(name="psY", bufs=1, space="PSUM"))

        w0m = consts.tile([P, 8 * 128], F32R)
        w1b = consts.tile([P, 128], BF16)
        w2m = consts.tile([P, 8 * 128], BF16)
        bv = consts.tile([P, 3], F32)
        nc.sync.dma_start(w0m[:], w0m_d[:])
        nc.sync.dma_start(w1b[:], w1b_d[:])
        nc.sync.dma_start(w2m[:], w2m_d[:])
        nc.sync.dma_start(bv[:], bv_d[:])
        b0vec = bv[:, 0:1]
        b1vec = bv[:, 1:2]
        b2vec = bv[:, 2:3]

        actsel = 0
        for s in range(nsuper):
            x_sb = xpool.tile([P, CTILE], F32R)
            nc.sync.dma_start(x_sb[:], x_d[:, s * CTILE : (s + 1) * CTILE])
            out_sb = opool.tile([P, CTILE], F32)
            for cp in range(NPAIR):
                p0 = cp * 1024
                x_pair = x_sb[:, p0 : p0 + 1024]
                y_ps = ypool.tile([P, 1024], F32)
                for i in range(8):
                    w0i = w0m[:, 128 * i : 128 * (i + 1)]
                    w2i = w2m[:, 128 * i : 128 * (i + 1)]
                    a_ps = apool.tile([P, 1024], F32)
                    for h in range(2):
                        nc.tensor.matmul(
                            a_ps[:, 512 * h : 512 * (h + 1)],
                            w0i,
                            x_sb[:, p0 + 512 * h : p0 + 512 * (h + 1)],
                            start=True,
                            stop=True,
                        )
                    h0 = h0pool.tile([P, 1024], BF16)
                    if actsel % 2 == 0:
                        nc.scalar.activation(h0[:], a_ps[:], RELU, bias=b0vec)
                    else:
                        nc.vector.tensor_scalar(
                            h0[:], a_ps[:], b0vec, 0.0, op0=ADD, op1=MAX
                        )
                    z_ps = bpool.tile([P, 1024], F32)
                    for h in range(2):
                        nc.tensor.matmul(
                            z_ps[:, 512 * h : 512 * (h + 1)],
                            w1b[:],
                            h0[:, 512 * h : 512 * (h + 1)],
                            start=True,
                            stop=True,
                        )
                    h1 = h1pool.tile([P, 1024], BF16)
                    if actsel % 2 == 0:
                        nc.vector.tensor_scalar(
                            h1[:], z_ps[:], b1vec, 0.0, op0=ADD, op1=MAX
                        )
                    else:
                        nc.scalar.activation(h1[:], z_ps[:], RELU, bias=b1vec)
                    actsel += 1
                    for h in range(2):
                        nc.tensor.matmul(
                            y_ps[:, 512 * h : 512 * (h + 1)],
                            w2i,
                            h1[:, 512 * h : 512 * (h + 1)],
                            start=(i == 0),
                            stop=(i == 7),
                        )
                # out = (y + b2) + x  -- exact fp32 residual add on VectorE
                nc.vector.scalar_tensor_tensor(
                    out_sb[:, p0 : p0 + 1024],
                    y_ps[:],
                    b2vec,
                    x_pair.bitcast(F32),
                    op0=ADD,
                    op1=ADD,
                )
            nc.sync.dma_start(out_d[:, s * CTILE : (s + 1) * CTILE], out_sb[:])

    nc.compile()
    return nc


def _get_nc(nsuper: int):
    if nsuper not in _BUILD_CACHE:
        _BUILD_CACHE[nsuper] = _build_nc(nsuper)
    return _BUILD_CACHE[nsuper]


def _make_weight_maps(W0, b0, W1, b1, W2, b2):
    """Host-side packing of the block stationary matrices."""
    import ml_dtypes

    W0v = np.asarray(W0, np.float32).reshape(8)
    b0v = np.asarray(b0, np.float32).reshape(8)
    W1m = np.asarray(W1, np.float32).reshape(8, 8)
    b1v = np.asarray(b1, np.float32).reshape(8)
    W2v = np.asarray(W2, np.float32).reshape(8)
    b2val = float(np.asarray(b2, np.float32).reshape(()))

    w0maps = np.zeros((P, 8 * 128), np.float32)
    w2maps = np.zeros((P, 8 * 128), np.float32)
    for i in range(8):
        for g in range(16):
            for j in range(8):
                # mm1: a[8g+j] += W0[j] * x[16i+g]
                w0maps[16 * i + g, 128 * i + 8 * g + j] = W0v[j]
                # mm3: y[16i+g] += W2[j] * h1[8g+j]
                w2maps[8 * g + j, 128 * i + 16 * i + g] = W2v[j]

    w1blk = np.zeros((P, 128), np.float32)
    for g in range(16):
        # mm2: z[8g+k] += W1[j,k] * h0[8g+j]  (lhsT[row=j_in, col=k_out])
        w1blk[8 * g : 8 * g + 8, 8 * g : 8 * g + 8] = W1m

    biasvecs = np.zeros((P, 3), np.float32)
    biasvecs[:, 0] = np.tile(b0v, 16)
    biasvecs[:, 1] = np.tile(b1v, 16)
    biasvecs[:, 2] = b2val
    return (
        w0maps,
        w1blk.astype(ml_dtypes.bfloat16),
        w2maps.astype(ml_dtypes.bfloat16),
        biasvecs,
    )


def _run(x, W0, b0, W1, b1, W2, b2, nsuper=NSUPER, trace=False):
    x = np.ascontiguousarray(np.asarray(x, np.float32))
    w0maps, w1blk, w2maps, biasvecs = _make_weight_maps(W0, b0, W1, b1, W2, b2)

    free = nsuper * CTILE
    n_el = N_CORES * P * free
    xs = x.reshape(-1)[:n_el].reshape(N_CORES, P, free)

    nc = _get_nc(nsuper)
    in_maps = [
        {
            "x": np.ascontiguousarray(xs[c]),
            "w0maps": w0maps,
            "w1blk": w1blk,
            "w2maps": w2maps,
            "biasvecs": biasvecs,
        }
        for c in range(N_CORES)
    ]
    res = run_bass_kernel_spmd(nc, in_maps, list(range(N_CORES)), trace=trace)
    out = np.stack([res.results[c]["out"] for c in range(N_CORES)])
    return out, res


def kernel(x, W0, b0, W1, b1, W2, b2):
    out, _ = _run(x, W0, b0, W1, b1, W2, b2)
    return out.reshape(FULL_SHAPE).astype(np.float32)
